# revision 10
# baseline (speedup 1.0000x reference)
"""EnergyTransformerLayer on 8 Trainium2 NeuronCores (Bass/Tile). v2

Sharding (per spec hint): heads are sharded across the 8 cores (2 heads each)
for the 5-step energy-descent loop; Q_opt is exchanged with an AllToAll before
the Wo projection; Wo + residual + FFN are sharded by target rows (128 rows
per core); the whole tail runs in TRANSPOSED space ([e, t] layouts) so no
on-device transposes are needed — the host transposes each core's [1024, 128]
output block.

Descent-loop engine split (per th-pass of 16 k-chunks):
  - MM1 (scores)  : PE, bf16, contraction 64 per head
  - exp           : ACT (Exp->fp8) for most chunks; fast-exp on DVE and
                    GPSIMD (affine f32->int8 bitcast as fp8e4m3, Schraudolph)
                    for the rest -- three engines compute exp concurrently
  - MM2 (grad+den): PE, fp8e4m3 DoubleRow over k-chunk pairs (2x contraction
                    per instruction, 0.5 cycles/row)
"""
import numpy as np
import ml_dtypes

import concourse.bass as bass
import concourse.mybir as mybir
import concourse.tile as tile
from concourse import bacc
from concourse.bass_utils import run_bass_kernel_spmd
from concourse.masks import make_identity

dt = mybir.dt
AF = mybir.ActivationFunctionType
ALU = mybir.AluOpType

N_CORES = 8
EMBED = 1024
N_HEADS = 16
HD = 64
HIDDEN = 4096
N_CTX = 2048
N_TGT = 1024
STEPS = 5
BETA = 1.0 / 8.0          # BETA / sqrt(HD)
INV_STEP = 10.0           # 1 / STEP_SIZE, folded into the ones-block of K_aug

HPC = N_HEADS // N_CORES  # heads per core = 2
TPC = N_TGT // N_CORES    # target rows per core = 128

BF = dt.bfloat16
F8 = dt.float8e4
F32 = dt.float32

# swappable for simulation (CoreSim implements no gelu variant)
GELU_FN = AF.Gelu_apprx_tanh
SPLIT_IN_DMA = True

DC = EMBED // 128     # 8 d-chunks
KC = N_CTX // 128     # 16 k-chunks
HC = HIDDEN // 128    # 32 hidden-chunks

# Schraudolph fast-exp: bits(e4m3(exp(x))) ~= x*8*log2(e) + 56 + c.
# DVE/GPSIMD converts f32->int8 by TRUNCATION, so add 0.5 for rounding.
SCHR_A = float(BETA * 8.0 / np.log(2.0))
SCHR_B = float(56.0 - 0.67 + 0.5)
# engine per k-chunk within a th-pass: ACT gets most, DVE offloads the rest.
# GPSIMD cannot read PSUM on HW, so it instead runs the SBUF-only descent
# ops (qT updates, qbf copies), freeing DVE cycles for fast-exp.
# DVE chunks are interleaved with ACT chunks (odd kc of the first 6 pairs)
# so the in-order PE stream never waits long on a single engine.
SCHR_DVE = frozenset({1, 3, 5, 7, 9, 11})
SCHR_POOL = frozenset()


def build_kernel(replicas: int = 1, no_collective: bool = False,
                 loop_n: int = 1, gate_weights: bool = True,
                 schr_dve=SCHR_DVE, schr_pool=SCHR_POOL):
    """Build the SPMD Bacc program (same NEFF on all 8 cores).

    no_collective=True replaces the AllToAll with a local DRAM copy — only
    for timing analysis (the A2A cost is excluded; output mixes t-blocks
    wrongly but is numerically representative).
    loop_n>1 wraps the body in a hardware For_i loop for slope timing.
    """
    nc = bacc.Bacc("TRN2", target_bir_lowering=False, debug=False,
                   num_devices=N_CORES)

    ctxT_d = nc.dram_tensor("ctxT", [EMBED, N_CTX], BF, kind="ExternalInput")
    tgtT_d = nc.dram_tensor("tgtT", [EMBED, N_TGT], BF, kind="ExternalInput")
    tgtTrows_d = nc.dram_tensor("tgtTrows", [EMBED, TPC], F32,
                                kind="ExternalInput")
    wqkT_d = nc.dram_tensor("wqkT", [EMBED, 2 * HPC * HD], BF, kind="ExternalInput")
    woT_d = nc.dram_tensor("woT", [EMBED, EMBED], BF, kind="ExternalInput")
    w1T_d = nc.dram_tensor("w1T", [EMBED, HIDDEN], BF, kind="ExternalInput")
    w2T_d = nc.dram_tensor("w2T", [HIDDEN, EMBED], BF, kind="ExternalInput")
    alphas_d = nc.dram_tensor("alphas", [128, 2], F32, kind="ExternalInput")
    out_d = nc.dram_tensor("outT", [EMBED, TPC], F32, kind="ExternalOutput")

    with tile.TileContext(nc) as tc:
        with (
            tc.tile_pool(name="const", bufs=1) as cpool,
            tc.tile_pool(name="persist", bufs=1) as pp,
            tc.tile_pool(name="wts", bufs=1) as wp,
            tc.tile_pool(name="stream", bufs=3) as sp,
            tc.tile_pool(name="work", bufs=1) as wk,
            tc.tile_pool(name="psA", bufs=3, space="PSUM") as psA,  # [128,1024]f32: 2 banks
            tc.tile_pool(name="psB", bufs=2, space="PSUM") as psB,  # [128,512]f32: 1 bank
            tc.tile_pool(name="dram", bufs=1, space="DRAM") as dp,
        ):
            alphas = cpool.tile([128, 2], F32)
            nc.sync.dma_start(out=alphas[:], in_=alphas_d[:])
            ident = cpool.tile([128, 128], BF)
            make_identity(nc, ident[:])

            wqkT = cpool.tile([128, DC * 256], BF)        # [d-chunk | wq128 wk128]
            nc.sync.dma_start(
                out=wqkT[:].rearrange("p (a f) -> p a f", a=DC),
                in_=wqkT_d.rearrange("(a p) f -> p a f", p=128),
            )
            woT_sb = wp.tile([128, DC * EMBED], BF)       # [d-chunk | e]

            def body(rep):
                # ------------- phase 1+2: tnorm, K / q projections ----------
                KT = pp.tile([128, N_CTX], BF, tag="KT", name=f"KT{rep}")
                Kaug = pp.tile([128, KC * 2 * 128], F8, tag="Kaug",
                               name=f"Kaug{rep}")
                nc.gpsimd.memset(Kaug[:], INV_STEP)
                qT = pp.tile([128, N_TGT], F32, tag="qT", name=f"qT{rep}")

                kps = [psA.tile([128, 1024], F32, tag="psA", name=f"kps{rep}_{i}")
                       for i in range(2)]
                qps = psA.tile([128, 1024], F32, tag="psA", name=f"qps{rep}")
                last_in_dma = None
                nsp = 2 if SPLIT_IN_DMA else 1
                for d in range(DC):
                    ctx_t = sp.tile([128, N_CTX], BF, tag="ctx", name=f"ctx{rep}_{d}")
                    cw = N_CTX // nsp
                    for hh in range(nsp):
                        last_in_dma = nc.sync.dma_start(
                            out=ctx_t[:, hh * cw:(hh + 1) * cw],
                            in_=ctxT_d.rearrange("(a p) k -> p a k", p=128)[
                                :, d, hh * cw:(hh + 1) * cw],
                        )
                    tgt_t = sp.tile([128, N_TGT], BF, tag="tgt", name=f"tgt{rep}_{d}")
                    tw = N_TGT // nsp
                    for hh in range(nsp):
                        nc.sync.dma_start(
                            out=tgt_t[:, hh * tw:(hh + 1) * tw],
                            in_=tgtT_d.rearrange("(a p) t -> p a t", p=128)[
                                :, d, hh * tw:(hh + 1) * tw],
                        )
                    tn_t = sp.tile([128, N_TGT], BF, tag="tn", name=f"tn{rep}_{d}")
                    nc.scalar.activation(tn_t[:], tgt_t[:], AF.Tanh,
                                         scale=alphas[:, 0:1])
                    wq = wqkT[:, d * 256:d * 256 + 128]
                    wkk = wqkT[:, d * 256 + 128:d * 256 + 256]
                    first, last = d == 0, d == DC - 1
                    for kcol in range(4):
                        nc.tensor.matmul(
                            kps[kcol // 2][:, (kcol % 2) * 512:(kcol % 2 + 1) * 512],
                            wkk, ctx_t[:, kcol * 512:(kcol + 1) * 512],
                            start=first, stop=last)
                    for tcol in range(2):
                        nc.tensor.matmul(
                            qps[:, tcol * 512:(tcol + 1) * 512],
                            wq, tn_t[:, tcol * 512:(tcol + 1) * 512],
                            start=first, stop=last)
                for i in range(2):
                    nc.vector.tensor_copy(
                        KT[:, i * 1024:(i + 1) * 1024], kps[i][:])
                nc.vector.tensor_copy(qT[:], qps[:])

                # transpose K_hT -> K_aug blocks ([k, z] layout per head, fp8)
                for kc in range(KC):
                    ktp = psB.tile([128, 128], BF, tag="psB", name=f"ktp{rep}_{kc}")
                    nc.tensor.transpose(ktp[:], KT[:, kc * 128:(kc + 1) * 128],
                                        ident[:])
                    base = kc * 256
                    nc.vector.tensor_copy(
                        Kaug[:, base:base + 256].rearrange(
                            "p (h f) -> p h f", f=128)[:, :, 0:64],
                        ktp[:].rearrange("p (h f) -> p h f", f=64),
                    )

                # FFN / Wo / residual-base streaming: early DMAs gated behind
                # the input ramp so ctx/tgt aren't contended.
                from concourse.tile import add_dep_helper

                gate = last_in_dma.ins

                def gated_dma(out, in_):
                    wd = nc.sync.dma_start(out=out, in_=in_)
                    if gate_weights:
                        add_dep_helper(wd.ins, gate, sync=True,
                                       reason="after ramp")
                    return wd

                tgtTrows = wp.tile([128, DC * TPC], F32, tag="tgtTrows",
                                   name=f"tgtTrows{rep}")
                gated_dma(
                    tgtTrows[:].rearrange("p (a t) -> p a t", a=DC),
                    tgtTrows_d.rearrange("(a p) t -> p a t", p=128),
                )
                for a in range(DC):
                    gated_dma(
                        woT_sb[:, a * EMBED:(a + 1) * EMBED],
                        woT_d.rearrange("(a p) e -> p a e", p=128)[:, a, :],
                    )
                w1cs, w2cs = [], []
                for q in range(4):
                    w1c = wp.tile([128, DC * 1024], BF, tag="w1s", bufs=2,
                                  name=f"w1c{rep}_{q}")
                    for a in range(DC):
                        gated_dma(
                            w1c[:, a * 1024:(a + 1) * 1024],
                            w1T_d.rearrange("(a p) h -> p a h", p=128)[
                                :, a, q * 1024:(q + 1) * 1024],
                        )
                    w1cs.append(w1c)
                for q in range(4):
                    w2c = wp.tile([128, 8 * EMBED], BF, tag="w2s", bufs=4,
                                  name=f"w2c{rep}_{q}")
                    for j in range(8):
                        hc = q * 8 + j
                        gated_dma(
                            w2c[:, j * EMBED:(j + 1) * EMBED],
                            w2T_d.rearrange("(a p) e -> p a e", p=128)[:, hc, :],
                        )
                    w2cs.append(w2c)

                # ------------- phase 3: 5-step energy descent ---------------
                qbf = {}
                for th in range(2):
                    tsl = slice(th * 512, (th + 1) * 512)
                    b = wk.tile([128, 512], BF, tag=f"qbf{th}", bufs=2,
                                name=f"qbf{rep}_init{th}")
                    nc.vector.tensor_copy(b[:], qT[:, tsl])
                    qbf[th] = b

                q_loc = dp.tile([N_CORES * 128, TPC], BF, name=f"qloc{rep}")
                q_ex = dp.tile([N_CORES * 128, TPC], BF, name=f"qex{rep}")

                for step in range(STEPS):
                    for th in range(2):
                        tsl = slice(th * 512, (th + 1) * 512)
                        upd = [psB.tile([128, 512], F32, tag="psB",
                                        name=f"upd{rep}_{step}_{th}_{h}")
                               for h in range(2)]
                        # MM2 lags MM1/exp by one kc-pair so the in-order PE
                        # stream never waits on the current pair's exp.
                        kav = Kaug[:].rearrange("p (kc h f) -> p kc h f",
                                                h=2, f=128)

                        def mm2(kcp, ex):
                            exv = ex[:].rearrange("p (k h t) -> p k h t",
                                                  k=2, h=2)
                            for h in range(2):
                                nc.tensor.matmul(
                                    upd[h][:],
                                    kav[:, 2 * kcp:2 * kcp + 2, h, :],
                                    exv[:, :, h, :],
                                    start=(kcp == 0), stop=(kcp == KC // 2 - 1),
                                    perf_mode=mybir.MatmulPerfMode.DoubleRow,
                                )

                        pend = None
                        for kcp in range(KC // 2):
                            ex = wk.tile([128, 2048], F8, tag="ex", bufs=3,
                                         name=f"ex{rep}_{step}_{th}_{kcp}")
                            for j in range(2):
                                kc = 2 * kcp + j
                                sc = psA.tile([128, 1024], F32, tag="psA",
                                              name=f"sc{rep}_{step}_{th}_{kc}")
                                for h in range(2):
                                    nc.tensor.matmul(
                                        sc[:, h * 512:(h + 1) * 512],
                                        KT[h * 64:(h + 1) * 64,
                                           kc * 128:(kc + 1) * 128],
                                        qbf[th][h * 64:(h + 1) * 64, :],
                                        start=True, stop=True,
                                    )
                                exj = ex[:, j * 1024:(j + 1) * 1024]
                                if kc in schr_dve:
                                    nc.vector.tensor_scalar(
                                        exj.bitcast(dt.int8), sc[:],
                                        SCHR_A, SCHR_B, ALU.mult, ALU.add)
                                elif kc in schr_pool:
                                    nc.gpsimd.tensor_scalar(
                                        exj.bitcast(dt.int8), sc[:],
                                        SCHR_A, SCHR_B, ALU.mult, ALU.add)
                                else:
                                    nc.scalar.activation(exj, sc[:], AF.Exp,
                                                         scale=BETA)
                            if pend is not None:
                                mm2(*pend)
                            pend = (kcp, ex)
                        mm2(*pend)
                        for h in range(2):
                            rec = wk.tile([128, 512], F32, tag="rec", bufs=2,
                                          name=f"rec{rep}_{step}_{th}_{h}")
                            nc.vector.reciprocal(rec[64:128, :], upd[h][64:128, :])
                            dq = wk.tile([128, 512], F32, tag="dq", bufs=2,
                                         name=f"dq{rep}_{step}_{th}_{h}")
                            hsl = slice(h * 64, (h + 1) * 64)
                            nc.vector.tensor_tensor(
                                dq[hsl, :], upd[h][0:64, :], rec[64:128, :],
                                ALU.mult,
                            )
                            nc.gpsimd.tensor_tensor(
                                qT[hsl, tsl], qT[hsl, tsl], dq[hsl, :],
                                ALU.add,
                            )
                        if step < STEPS - 1:
                            b = wk.tile([128, 512], BF, tag=f"qbf{th}", bufs=2,
                                        name=f"qbf{rep}_{step}_{th}")
                            nc.gpsimd.tensor_copy(b[:], qT[:, tsl])
                            qbf[th] = b
                        else:
                            # last step: stage this t-half to DRAM for the
                            # AllToAll while the other half still computes
                            qfin = wk.tile([128, 512], BF, tag=f"qbf{th}",
                                           bufs=2, name=f"qfin{rep}_{th}")
                            nc.gpsimd.tensor_copy(qfin[:], qT[:, tsl])
                            nc.sync.dma_start(
                                out=q_loc[th * 512:(th + 1) * 512, :]
                                .rearrange("(j p) t -> p j t", p=128),
                                in_=qfin[:].rearrange("p (j t) -> p j t", j=4),
                            )

                # Keep the PE p-state ramped through the A2A wait: dummy
                # matmuls with no A2A dependency run during the collective,
                # so the tail's matmuls start at full clock instead of
                # re-ramping from idle.
                for wi in range(12):
                    warm = psB.tile([128, 512], F32, tag="psB",
                                    name=f"warm{rep}_{wi}")
                    nc.tensor.matmul(warm[:], ident[:], KT[:, 0:512],
                                     start=True, stop=True)

                # ------------- phase 4: AllToAll on Q -----------------------
                if no_collective:
                    nc.sync.dma_start(out=q_ex[:], in_=q_loc[:])
                else:
                    nc.gpsimd.collective_compute(
                        "AllToAll",
                        ALU.bypass,
                        replica_groups=[list(range(N_CORES))],
                        ins=[q_loc[:]],
                        outs=[q_ex[:]],
                    )
                qto = wk.tile([128, DC * TPC], BF, tag="qto", name=f"qto{rep}")
                nc.sync.dma_start(
                    out=qto[:].rearrange("p (a t) -> p a t", a=DC),
                    in_=q_ex[:].rearrange("(a p) t -> p a t", p=128),
                )

                # ------------- phase 5: Wo^T + residual (transposed) --------
                # atnT[e, t] = Wo @ q_opt^T : lhsT = woT chunks [d, e-cols]
                atnT = psA.tile([128, 1024], F32, tag="psA", name=f"atnT{rep}")
                for ec in range(8):
                    for a in range(DC):
                        nc.tensor.matmul(
                            atnT[:, ec * 128:(ec + 1) * 128],
                            woT_sb[:, a * EMBED + ec * 128:
                                   a * EMBED + (ec + 1) * 128],
                            qto[:, a * TPC:(a + 1) * TPC],
                            start=(a == 0), stop=(a == DC - 1),
                        )
                t2T = pp.tile([128, DC * TPC], F32, tag="t2T", name=f"t2T{rep}")
                nc.vector.tensor_tensor(t2T[:], tgtTrows[:], atnT[:], ALU.add)
                t2nT = wk.tile([128, DC * TPC], BF, tag="t2nT", name=f"t2nT{rep}")
                nc.scalar.activation(t2nT[:], t2T[:], AF.Tanh,
                                     scale=alphas[:, 1:2])

                # ------------- phase 6: FFN (transposed) --------------------
                # FFN1 rounds (gelu overlaps next round), then FFN2 with one
                # psum accumulation group open at a time (zero-region rule).
                GT2 = wk.tile([128, HIDDEN], BF, tag="GT2", name=f"GT2{rep}")
                for r in range(4):
                    hps = psA.tile([128, 1024], F32, tag="psA",
                                   name=f"hps{rep}_{r}")
                    w1c = w1cs[r]
                    for hcj in range(8):
                        for a in range(DC):
                            nc.tensor.matmul(
                                hps[:, hcj * 128:(hcj + 1) * 128],
                                w1c[:, a * 1024 + hcj * 128:
                                    a * 1024 + (hcj + 1) * 128],
                                t2nT[:, a * TPC:(a + 1) * TPC],
                                start=(a == 0), stop=(a == DC - 1),
                            )
                    nc.scalar.activation(
                        GT2[:, r * 1024:(r + 1) * 1024], hps[:], GELU_FN)
                ffnT = psA.tile([128, 1024], F32, tag="psA", name=f"ffnT{rep}")
                for ec in range(8):
                    for hc in range(HC):
                        w2c = w2cs[hc // 8]
                        nc.tensor.matmul(
                            ffnT[:, ec * 128:(ec + 1) * 128],
                            w2c[:, (hc % 8) * EMBED + ec * 128:
                                (hc % 8) * EMBED + (ec + 1) * 128],
                            GT2[:, hc * 128:(hc + 1) * 128],
                            start=(hc == 0), stop=(hc == HC - 1),
                        )
                outT = wk.tile([128, DC * TPC], F32, tag="outT", name=f"outT{rep}")
                nc.vector.tensor_tensor(outT[:], t2T[:], ffnT[:], ALU.add)
                nc.sync.dma_start(
                    out=out_d.rearrange("(a p) t -> p a t", p=128),
                    in_=outT[:].rearrange("p (a t) -> p a t", a=DC),
                )

            if loop_n > 1:
                assert no_collective and replicas == 1
                with tc.For_i(0, loop_n, 1):
                    body(0)
            else:
                for rep in range(replicas):
                    body(rep)

    nc.compile()
    return nc


def prepare_inputs(context, target, Wq, Wk, Wo, W1, W2, alpha1, alpha2):
    """Per-core host-side layout prep. Returns list of 8 in_maps."""
    bf = ml_dtypes.bfloat16
    context = np.asarray(context, np.float32)
    target = np.asarray(target, np.float32)
    ctxT = np.ascontiguousarray(context.T).astype(bf)            # [1024, 2048]
    tgtT_f = np.ascontiguousarray(target.T)                      # [1024, 1024] f32
    tgtT = tgtT_f.astype(bf)
    woT = np.ascontiguousarray(np.asarray(Wo, np.float32).T).astype(bf)
    w1T = np.ascontiguousarray(np.asarray(W1, np.float32).T).astype(bf)
    w2T = np.ascontiguousarray(np.asarray(W2, np.float32).T).astype(bf)
    alphas = np.zeros((128, 2), np.float32)
    alphas[:, 0] = np.float32(np.asarray(alpha1).reshape(-1)[0])
    alphas[:, 1] = np.float32(np.asarray(alpha2).reshape(-1)[0])
    Wq = np.asarray(Wq, np.float32)
    Wk = np.asarray(Wk, np.float32)

    in_maps = []
    for c in range(N_CORES):
        hs = slice(c * HPC, (c + 1) * HPC)
        wq = Wq[hs].reshape(HPC * HD, EMBED)
        wkk = Wk[hs].reshape(HPC * HD, EMBED)
        wqkT = np.concatenate(
            [np.ascontiguousarray(wq.T), np.ascontiguousarray(wkk.T)], axis=1
        ).astype(bf)                                             # [1024, 256]
        in_maps.append({
            "ctxT": ctxT,
            "tgtT": tgtT,
            "tgtTrows": np.ascontiguousarray(
                tgtT_f[:, c * TPC:(c + 1) * TPC]).astype(np.float32),
            "wqkT": wqkT,
            "woT": woT,
            "w1T": w1T,
            "w2T": w2T,
            "alphas": alphas,
        })
    return in_maps


def kernel(context, target, Wq, Wk, Wo, W1, W2, alpha1, alpha2):
    in_maps = prepare_inputs(context, target, Wq, Wk, Wo, W1, W2,
                             alpha1, alpha2)
    nc = build_kernel()
    res = run_bass_kernel_spmd(nc, in_maps, list(range(N_CORES)))
    out = np.concatenate(
        [np.ascontiguousarray(res.results[c]["outT"].T)
         for c in range(N_CORES)], axis=0
    )
    return out.astype(np.float32)


# revision 18
# speedup vs baseline: 1.2165x; 1.2165x over previous
"""EnergyTransformerLayer on 8 Trainium2 NeuronCores (Bass/Tile). v2

Sharding (per spec hint): heads are sharded across the 8 cores (2 heads each)
for the 5-step energy-descent loop; Q_opt is exchanged with an AllToAll before
the Wo projection; Wo + residual + FFN are sharded by target rows (128 rows
per core); the whole tail runs in TRANSPOSED space ([e, t] layouts) so no
on-device transposes are needed — the host transposes each core's [1024, 128]
output block.

Descent-loop engine split (per th-pass of 16 k-chunks):
  - MM1 (scores)  : PE, bf16, contraction 64 per head
  - exp           : ACT (Exp->fp8) for most chunks; fast-exp on DVE and
                    GPSIMD (affine f32->int8 bitcast as fp8e4m3, Schraudolph)
                    for the rest -- three engines compute exp concurrently
  - MM2 (grad+den): PE, fp8e4m3 DoubleRow over k-chunk pairs (2x contraction
                    per instruction, 0.5 cycles/row)
"""
import numpy as np
import ml_dtypes

import concourse.bass as bass
import concourse.mybir as mybir
import concourse.tile as tile
from concourse import bacc
from concourse.bass_utils import run_bass_kernel_spmd
from concourse.masks import make_identity

dt = mybir.dt
AF = mybir.ActivationFunctionType
ALU = mybir.AluOpType

N_CORES = 8
EMBED = 1024
N_HEADS = 16
HD = 64
HIDDEN = 4096
N_CTX = 2048
N_TGT = 1024
STEPS = 5
BETA = 1.0 / 8.0          # BETA / sqrt(HD)
INV_STEP = 10.0           # 1 / STEP_SIZE, folded into the ones-block of K_aug

HPC = N_HEADS // N_CORES  # heads per core = 2
TPC = N_TGT // N_CORES    # target rows per core = 128

BF = dt.bfloat16
F8 = dt.float8e4
F32 = dt.float32

# swappable for simulation (CoreSim implements no gelu variant)
GELU_FN = AF.Gelu_apprx_tanh
SPLIT_IN_DMA = True

DC = EMBED // 128     # 8 d-chunks
KC = N_CTX // 128     # 16 k-chunks
HC = HIDDEN // 128    # 32 hidden-chunks

# Schraudolph fast-exp: bits(e4m3(exp(x))) ~= x*8*log2(e) + 56 + c.
# DVE/GPSIMD converts f32->int8 by TRUNCATION, so add 0.5 for rounding.
SCHR_A = float(BETA * 8.0 / np.log(2.0))
SCHR_B = float(56.0 - 0.67 + 0.5)
# engine per k-chunk within a th-pass: ACT gets most, DVE offloads the rest.
# GPSIMD cannot read PSUM on HW, so it instead runs the SBUF-only descent
# ops (qT updates, qbf copies), freeing DVE cycles for fast-exp.
# DVE chunks sit late in each pass (odd kc of the last 6 pairs): the first
# chunks are all-ACT, so the previous pass's DVE update chain (rec/mult)
# drains before the first fast-exp chunk is due.
SCHR_DVE = frozenset({5, 7, 9, 11, 13, 15})
SCHR_POOL = frozenset()


def build_kernel(replicas: int = 1, no_collective: bool = False,
                 loop_n: int = 1, gate_weights: bool = True,
                 schr_dve=SCHR_DVE, schr_pool=SCHR_POOL):
    """Build the SPMD Bacc program (same NEFF on all 8 cores).

    no_collective=True replaces the AllToAll with a local DRAM copy — only
    for timing analysis (the A2A cost is excluded; output mixes t-blocks
    wrongly but is numerically representative).
    loop_n>1 wraps the body in a hardware For_i loop for slope timing.
    """
    nc = bacc.Bacc("TRN2", target_bir_lowering=False, debug=False,
                   num_devices=N_CORES)

    ctxT_d = nc.dram_tensor("ctxT", [EMBED, N_CTX], F8, kind="ExternalInput")
    tgtT_d = nc.dram_tensor("tgtT", [EMBED, N_TGT], BF, kind="ExternalInput")
    tgtTrows_d = nc.dram_tensor("tgtTrows", [EMBED, TPC], F32,
                                kind="ExternalInput")
    wqT_d = nc.dram_tensor("wqT", [EMBED, HPC * HD], BF, kind="ExternalInput")
    wk8_d = nc.dram_tensor("wk8", [EMBED, HPC * HD], F8, kind="ExternalInput")
    woT_d = nc.dram_tensor("woT", [EMBED, EMBED], BF, kind="ExternalInput")
    w1T_d = nc.dram_tensor("w1T", [EMBED, HIDDEN], BF, kind="ExternalInput")
    w2T_d = nc.dram_tensor("w2T", [HIDDEN, EMBED], BF, kind="ExternalInput")
    alphas_d = nc.dram_tensor("alphas", [128, 2], F32, kind="ExternalInput")
    out_d = nc.dram_tensor("outT", [EMBED, TPC], F32, kind="ExternalOutput")

    with tile.TileContext(nc) as tc:
        with (
            tc.tile_pool(name="const", bufs=1) as cpool,
            tc.tile_pool(name="persist", bufs=1) as pp,
            tc.tile_pool(name="wts", bufs=1) as wp,
            tc.tile_pool(name="stream", bufs=3) as sp,
            tc.tile_pool(name="work", bufs=1) as wk,
            tc.tile_pool(name="psA", bufs=3, space="PSUM") as psA,  # [128,1024]f32: 2 banks
            tc.tile_pool(name="psB", bufs=2, space="PSUM") as psB,  # [128,512]f32: 1 bank
            tc.tile_pool(name="dram", bufs=1, space="DRAM") as dp,
        ):
            alphas = cpool.tile([128, 2], F32)
            nc.sync.dma_start(out=alphas[:], in_=alphas_d[:])
            ident = cpool.tile([128, 128], BF)
            make_identity(nc, ident[:])

            wqT = cpool.tile([128, DC * 128], BF)         # [d-chunk | wq128]
            nc.sync.dma_start(
                out=wqT[:].rearrange("p (a f) -> p a f", a=DC),
                in_=wqT_d.rearrange("(a p) f -> p a f", p=128),
            )
            wk8 = cpool.tile([128, 4 * 2 * 128], F8)      # [d-pair | i | z]
            nc.sync.dma_start(
                out=wk8[:].rearrange("p (dp i m) -> p dp i m", dp=4, i=2),
                in_=wk8_d.rearrange("(dp i p) m -> p dp i m", p=128, i=2),
            )
            woT_sb = wp.tile([128, DC * EMBED], BF)       # [d-chunk | e]

            def body(rep):
                # ------------- phase 1+2: tnorm, K / q projections ----------
                KT = pp.tile([128, N_CTX], BF, tag="KT", name=f"KT{rep}")
                Kaug = pp.tile([128, KC * 2 * 128], F8, tag="Kaug",
                               name=f"Kaug{rep}")
                nc.gpsimd.memset(Kaug[:], INV_STEP)
                qT = pp.tile([128, N_TGT], F32, tag="qT", name=f"qT{rep}")

                kps = [psA.tile([128, 1024], F32, tag="psA", name=f"kps{rep}_{i}")
                       for i in range(2)]
                qps = psA.tile([128, 1024], F32, tag="psA", name=f"qps{rep}")
                # K projection: fp8 ctx + fp8 Wk, DoubleRow over d-pairs.
                # ctx DMAs are issued first — the K path (proj + transposes)
                # is the longer pole of the ramp.
                ctx_ts = []
                for dpr in range(4):
                    ctx_t = sp.tile([128, 2 * N_CTX], F8, tag="ctx",
                                    bufs=4, name=f"ctx{rep}_{dpr}")
                    for i in range(2):
                        nc.sync.dma_start(
                            out=ctx_t[:, i * N_CTX:(i + 1) * N_CTX],
                            in_=ctxT_d.rearrange("(dp i p) k -> p dp i k",
                                                 p=128, i=2)[:, dpr, i, :],
                        )
                    ctx_ts.append(ctx_t)
                for dpr in range(4):
                    ctxv = ctx_ts[dpr][:].rearrange("p (i k) -> p i k", i=2)
                    for kcol in range(4):
                        nc.tensor.matmul(
                            kps[kcol // 2][:, (kcol % 2) * 512:(kcol % 2 + 1) * 512],
                            wk8[:].rearrange("p (dp i m) -> p dp i m",
                                             dp=4, i=2)[:, dpr, :, :],
                            ctxv[:, :, kcol * 512:(kcol + 1) * 512],
                            start=(dpr == 0), stop=(dpr == 3),
                            perf_mode=mybir.MatmulPerfMode.DoubleRow,
                        )
                last_in_dma = None
                nsp = 2 if SPLIT_IN_DMA else 1
                for d in range(DC):
                    tgt_t = sp.tile([128, N_TGT], BF, tag="tgt", name=f"tgt{rep}_{d}")
                    tw = N_TGT // nsp
                    for hh in range(nsp):
                        last_in_dma = nc.sync.dma_start(
                            out=tgt_t[:, hh * tw:(hh + 1) * tw],
                            in_=tgtT_d.rearrange("(a p) t -> p a t", p=128)[
                                :, d, hh * tw:(hh + 1) * tw],
                        )
                    tn_t = sp.tile([128, N_TGT], BF, tag="tn", name=f"tn{rep}_{d}")
                    nc.scalar.activation(tn_t[:], tgt_t[:], AF.Tanh,
                                         scale=alphas[:, 0:1])
                    wq = wqT[:, d * 128:(d + 1) * 128]
                    for tcol in range(2):
                        nc.tensor.matmul(
                            qps[:, tcol * 512:(tcol + 1) * 512],
                            wq, tn_t[:, tcol * 512:(tcol + 1) * 512],
                            start=(d == 0), stop=(d == DC - 1))
                for i in range(2):
                    nc.vector.tensor_copy(
                        KT[:, i * 1024:(i + 1) * 1024], kps[i][:])
                nc.vector.tensor_copy(qT[:], qps[:])

                # transpose K_hT -> K_aug blocks ([k, z] layout per head, fp8)
                for kc in range(KC):
                    ktp = psB.tile([128, 128], BF, tag="psB", name=f"ktp{rep}_{kc}")
                    nc.tensor.transpose(ktp[:], KT[:, kc * 128:(kc + 1) * 128],
                                        ident[:])
                    base = kc * 256
                    nc.vector.tensor_copy(
                        Kaug[:, base:base + 256].rearrange(
                            "p (h f) -> p h f", f=128)[:, :, 0:64],
                        ktp[:].rearrange("p (h f) -> p h f", f=64),
                    )

                # FFN / Wo / residual-base streaming: early DMAs gated behind
                # the input ramp so ctx/tgt aren't contended.
                from concourse.tile import add_dep_helper

                gate = last_in_dma.ins

                def gated_dma(out, in_):
                    wd = nc.sync.dma_start(out=out, in_=in_)
                    if gate_weights:
                        add_dep_helper(wd.ins, gate, sync=True,
                                       reason="after ramp")
                    return wd

                tgtTrows = wp.tile([128, DC * TPC], F32, tag="tgtTrows",
                                   name=f"tgtTrows{rep}")
                gated_dma(
                    tgtTrows[:].rearrange("p (a t) -> p a t", a=DC),
                    tgtTrows_d.rearrange("(a p) t -> p a t", p=128),
                )
                for a in range(DC):
                    gated_dma(
                        woT_sb[:, a * EMBED:(a + 1) * EMBED],
                        woT_d.rearrange("(a p) e -> p a e", p=128)[:, a, :],
                    )
                w1cs, w2cs = [], []
                for q in range(4):
                    w1c = wp.tile([128, DC * 1024], BF, tag="w1s", bufs=2,
                                  name=f"w1c{rep}_{q}")
                    for a in range(DC):
                        gated_dma(
                            w1c[:, a * 1024:(a + 1) * 1024],
                            w1T_d.rearrange("(a p) h -> p a h", p=128)[
                                :, a, q * 1024:(q + 1) * 1024],
                        )
                    w1cs.append(w1c)
                for q in range(4):
                    w2c = wp.tile([128, 8 * EMBED], BF, tag="w2s", bufs=4,
                                  name=f"w2c{rep}_{q}")
                    for j in range(8):
                        hc = q * 8 + j
                        gated_dma(
                            w2c[:, j * EMBED:(j + 1) * EMBED],
                            w2T_d.rearrange("(a p) e -> p a e", p=128)[:, hc, :],
                        )
                    w2cs.append(w2c)

                # ------------- phase 3: 5-step energy descent ---------------
                qbf = {}
                for th in range(2):
                    tsl = slice(th * 512, (th + 1) * 512)
                    b = wk.tile([128, 512], BF, tag=f"qbf{th}", bufs=2,
                                name=f"qbf{rep}_init{th}")
                    nc.vector.tensor_copy(b[:], qT[:, tsl])
                    qbf[th] = b

                q_loc = dp.tile([N_CORES * 128, TPC], BF, name=f"qloc{rep}")
                q_ex = dp.tile([N_CORES * 128, TPC], BF, name=f"qex{rep}")

                for step in range(STEPS):
                    for th in range(2):
                        tsl = slice(th * 512, (th + 1) * 512)
                        upd = [psB.tile([128, 512], F32, tag="psB",
                                        name=f"upd{rep}_{step}_{th}_{h}")
                               for h in range(2)]
                        # MM2 lags MM1/exp by one kc-pair so the in-order PE
                        # stream never waits on the current pair's exp.
                        kav = Kaug[:].rearrange("p (kc h f) -> p kc h f",
                                                h=2, f=128)

                        def mm2(kcp, ex):
                            exv = ex[:].rearrange("p (k h t) -> p k h t",
                                                  k=2, h=2)
                            for h in range(2):
                                nc.tensor.matmul(
                                    upd[h][:],
                                    kav[:, 2 * kcp:2 * kcp + 2, h, :],
                                    exv[:, :, h, :],
                                    start=(kcp == 0), stop=(kcp == KC // 2 - 1),
                                    perf_mode=mybir.MatmulPerfMode.DoubleRow,
                                )

                        pend = None
                        for kcp in range(KC // 2):
                            ex = wk.tile([128, 2048], F8, tag="ex", bufs=3,
                                         name=f"ex{rep}_{step}_{th}_{kcp}")
                            for j in range(2):
                                kc = 2 * kcp + j
                                sc = psA.tile([128, 1024], F32, tag="psA",
                                              name=f"sc{rep}_{step}_{th}_{kc}")
                                for h in range(2):
                                    nc.tensor.matmul(
                                        sc[:, h * 512:(h + 1) * 512],
                                        KT[h * 64:(h + 1) * 64,
                                           kc * 128:(kc + 1) * 128],
                                        qbf[th][h * 64:(h + 1) * 64, :],
                                        start=True, stop=True,
                                    )
                                exj = ex[:, j * 1024:(j + 1) * 1024]
                                if kc in schr_dve:
                                    nc.vector.tensor_scalar(
                                        exj.bitcast(dt.int8), sc[:],
                                        SCHR_A, SCHR_B, ALU.mult, ALU.add)
                                elif kc in schr_pool:
                                    nc.gpsimd.tensor_scalar(
                                        exj.bitcast(dt.int8), sc[:],
                                        SCHR_A, SCHR_B, ALU.mult, ALU.add)
                                else:
                                    nc.scalar.activation(exj, sc[:], AF.Exp,
                                                         scale=BETA)
                            if pend is not None:
                                mm2(*pend)
                            pend = (kcp, ex)
                        mm2(*pend)
                        for h in range(2):
                            rec = wk.tile([128, 512], F32, tag="rec", bufs=2,
                                          name=f"rec{rep}_{step}_{th}_{h}")
                            nc.vector.reciprocal(rec[64:128, :], upd[h][64:128, :])
                            dq = wk.tile([128, 512], F32, tag="dq", bufs=2,
                                         name=f"dq{rep}_{step}_{th}_{h}")
                            hsl = slice(h * 64, (h + 1) * 64)
                            nc.vector.tensor_tensor(
                                dq[hsl, :], upd[h][0:64, :], rec[64:128, :],
                                ALU.mult,
                            )
                            nc.gpsimd.tensor_tensor(
                                qT[hsl, tsl], qT[hsl, tsl], dq[hsl, :],
                                ALU.add,
                            )
                        if step < STEPS - 1:
                            b = wk.tile([128, 512], BF, tag=f"qbf{th}", bufs=2,
                                        name=f"qbf{rep}_{step}_{th}")
                            nc.gpsimd.tensor_copy(b[:], qT[:, tsl])
                            qbf[th] = b
                        else:
                            # last step: stage this t-half to DRAM for the
                            # AllToAll while the other half still computes
                            qfin = wk.tile([128, 512], BF, tag=f"qbf{th}",
                                           bufs=2, name=f"qfin{rep}_{th}")
                            nc.gpsimd.tensor_copy(qfin[:], qT[:, tsl])
                            nc.sync.dma_start(
                                out=q_loc[th * 512:(th + 1) * 512, :]
                                .rearrange("(j p) t -> p j t", p=128),
                                in_=qfin[:].rearrange("p (j t) -> p j t", j=4),
                            )

                # ------------- phase 4: AllToAll on Q -----------------------
                if no_collective:
                    nc.sync.dma_start(out=q_ex[:], in_=q_loc[:])
                else:
                    nc.gpsimd.collective_compute(
                        "AllToAll",
                        ALU.bypass,
                        replica_groups=[list(range(N_CORES))],
                        ins=[q_loc[:]],
                        outs=[q_ex[:]],
                    )
                qto = wk.tile([128, DC * TPC], BF, tag="qto", name=f"qto{rep}")
                nc.sync.dma_start(
                    out=qto[:].rearrange("p (a t) -> p a t", a=DC),
                    in_=q_ex[:].rearrange("(a p) t -> p a t", p=128),
                )

                # ------------- phase 5: Wo^T + residual (transposed) --------
                # atnT[e, t] = Wo @ q_opt^T : lhsT = woT chunks [d, e-cols]
                atnT = psA.tile([128, 1024], F32, tag="psA", name=f"atnT{rep}")
                for ec in range(8):
                    for a in range(DC):
                        nc.tensor.matmul(
                            atnT[:, ec * 128:(ec + 1) * 128],
                            woT_sb[:, a * EMBED + ec * 128:
                                   a * EMBED + (ec + 1) * 128],
                            qto[:, a * TPC:(a + 1) * TPC],
                            start=(a == 0), stop=(a == DC - 1),
                        )
                t2T = pp.tile([128, DC * TPC], F32, tag="t2T", name=f"t2T{rep}")
                nc.vector.tensor_tensor(t2T[:], tgtTrows[:], atnT[:], ALU.add)
                t2nT = wk.tile([128, DC * TPC], BF, tag="t2nT", name=f"t2nT{rep}")
                nc.scalar.activation(t2nT[:], t2T[:], AF.Tanh,
                                     scale=alphas[:, 1:2])

                # ------------- phase 6: FFN (transposed) --------------------
                # FFN1 rounds (gelu overlaps next round), then FFN2 with one
                # psum accumulation group open at a time (zero-region rule).
                GT2 = wk.tile([128, HIDDEN], BF, tag="GT2", name=f"GT2{rep}")
                for r in range(4):
                    hps = psA.tile([128, 1024], F32, tag="psA",
                                   name=f"hps{rep}_{r}")
                    w1c = w1cs[r]
                    for hcj in range(8):
                        for a in range(DC):
                            nc.tensor.matmul(
                                hps[:, hcj * 128:(hcj + 1) * 128],
                                w1c[:, a * 1024 + hcj * 128:
                                    a * 1024 + (hcj + 1) * 128],
                                t2nT[:, a * TPC:(a + 1) * TPC],
                                start=(a == 0), stop=(a == DC - 1),
                            )
                    nc.scalar.activation(
                        GT2[:, r * 1024:(r + 1) * 1024], hps[:], GELU_FN)
                ffnT = psA.tile([128, 1024], F32, tag="psA", name=f"ffnT{rep}")
                for ec in range(8):
                    for hc in range(HC):
                        w2c = w2cs[hc // 8]
                        nc.tensor.matmul(
                            ffnT[:, ec * 128:(ec + 1) * 128],
                            w2c[:, (hc % 8) * EMBED + ec * 128:
                                (hc % 8) * EMBED + (ec + 1) * 128],
                            GT2[:, hc * 128:(hc + 1) * 128],
                            start=(hc == 0), stop=(hc == HC - 1),
                        )
                outT = wk.tile([128, DC * TPC], F32, tag="outT", name=f"outT{rep}")
                nc.vector.tensor_tensor(outT[:], t2T[:], ffnT[:], ALU.add)
                nc.sync.dma_start(
                    out=out_d.rearrange("(a p) t -> p a t", p=128),
                    in_=outT[:].rearrange("p (a t) -> p a t", a=DC),
                )

            if loop_n > 1:
                assert no_collective and replicas == 1
                with tc.For_i(0, loop_n, 1):
                    body(0)
            else:
                for rep in range(replicas):
                    body(rep)

    nc.compile()
    return nc


def prepare_inputs(context, target, Wq, Wk, Wo, W1, W2, alpha1, alpha2):
    """Per-core host-side layout prep. Returns list of 8 in_maps."""
    bf = ml_dtypes.bfloat16
    f8 = ml_dtypes.float8_e4m3
    context = np.asarray(context, np.float32)
    target = np.asarray(target, np.float32)
    ctxT = np.ascontiguousarray(context.T).astype(f8)            # [1024, 2048]
    tgtT_f = np.ascontiguousarray(target.T)                      # [1024, 1024] f32
    tgtT = tgtT_f.astype(bf)
    woT = np.ascontiguousarray(np.asarray(Wo, np.float32).T).astype(bf)
    w1T = np.ascontiguousarray(np.asarray(W1, np.float32).T).astype(bf)
    w2T = np.ascontiguousarray(np.asarray(W2, np.float32).T).astype(bf)
    alphas = np.zeros((128, 2), np.float32)
    alphas[:, 0] = np.float32(np.asarray(alpha1).reshape(-1)[0])
    alphas[:, 1] = np.float32(np.asarray(alpha2).reshape(-1)[0])
    Wq = np.asarray(Wq, np.float32)
    Wk = np.asarray(Wk, np.float32)

    in_maps = []
    for c in range(N_CORES):
        hs = slice(c * HPC, (c + 1) * HPC)
        wq = Wq[hs].reshape(HPC * HD, EMBED)
        wkk = Wk[hs].reshape(HPC * HD, EMBED)
        in_maps.append({
            "ctxT": ctxT,
            "tgtT": tgtT,
            "tgtTrows": np.ascontiguousarray(
                tgtT_f[:, c * TPC:(c + 1) * TPC]).astype(np.float32),
            "wqT": np.ascontiguousarray(wq.T).astype(bf),        # [1024, 128]
            "wk8": np.ascontiguousarray(wkk.T).astype(f8),       # [1024, 128]
            "woT": woT,
            "w1T": w1T,
            "w2T": w2T,
            "alphas": alphas,
        })
    return in_maps


def kernel(context, target, Wq, Wk, Wo, W1, W2, alpha1, alpha2):
    in_maps = prepare_inputs(context, target, Wq, Wk, Wo, W1, W2,
                             alpha1, alpha2)
    nc = build_kernel()
    res = run_bass_kernel_spmd(nc, in_maps, list(range(N_CORES)))
    out = np.concatenate(
        [np.ascontiguousarray(res.results[c]["outT"].T)
         for c in range(N_CORES)], axis=0
    )
    return out.astype(np.float32)
